# revision 24
# baseline (speedup 1.0000x reference)
"""Trainium2 Bass kernel for the BSG word2gauss-style hinge/KL loss.

Strategy (data-parallel over 8 NeuronCores), v2:
  - The type-means dot product is folded through the linear layer on host:
      mu.m_c = h.(Wmu @ m_c) + bmu.m_c
    so each vocab row only needs  W'_c = -2*Wmu@m_c  (50 wide) instead of
    m_c (100 wide), plus fused scalars  s1_c = sum(m_c^2) - 2*bmu.m_c,
    lv_c, iv_c = exp(-lv_c).  Table rows shrink to 128 bf16 (256B):
      GT [V,128]: cols 0:50 = U = emb@W1[:50], 50:100 = W', 100 = s1,
                  101 = lv, 102 = iv
      CU [V,128]: cols 0:50 = Ucen = emb@W1[50:]+b1, rest same as GT.
  - dma_gather's int16 index limit is handled by gathering PAIRED rows
    (idx = id>>1, elem = 512B) + a u32 parity select.  512B descriptors
    run at full DMA-engine rate, so pairing costs no transfer time.
  - ctx and neg share the GT table, so their gathers merge into ONE index
    stream per gather block: 10240 idxs in ten 1024-idx gathers (1024 is a
    hard SWDGE per-instruction limit) + one 512-idx cen gather, spread
    round-robin over the 4 SWDGE queues.  The wall-clock bottleneck is the
    GpSimd/Q7 descriptor generation (~2.4ns/idx, serial on the Pool engine)
    overlapped with SDMA descriptor processing (~25ns/desc over 16 engines).
  - Per gather block (512 rows x [10 ctx + 10 neg + 1 cen]):
      h = sum_j relu(U[ctx_j]+Ucen[cen])  (TT tree-add), mu/logsig via PE,
      dot = h.W' via one big TT mult + segmented reduce, then the KL/hinge
      algebra on [128,4,20] tiles.  Per-core partials [128,2] -> host.
"""

import sys

for _p in ("/opt/trn_rl_repo", "/opt/pypackages"):
    if _p not in sys.path:
        sys.path.append(_p)

from contextlib import ExitStack

import numpy as np
import ml_dtypes

import concourse.bass as bass
import concourse.tile as tile
from concourse import bacc, mybir
from concourse.bass_utils import run_bass_kernel_spmd
from concourse.masks import make_identity

dt = mybir.dt
F32 = dt.float32
BF16 = dt.bfloat16
U32 = dt.uint32
AF = mybir.ActivationFunctionType
OP = mybir.AluOpType
AX = mybir.AxisListType

V, D, H, L = 50000, 50, 50, 100
C = 10
B = 65536
NCORES = 8
NB = B // NCORES     # rows per core: 8192
GBS = 512            # rows per gather block
NGB = NB // GBS      # 16
NSB = GBS // 128     # 4 sub-blocks
SLOT = 2 * C         # 20 merged (ctx|neg) slots per sub-block
MQ = NSB * SLOT      # 80 merged slots per gather block
MN = MQ * 128        # 10240 merged idxs per gather block
ZN = NSB * 128       # 512 cen idxs per gather block
E = 128              # table row width (bf16 elems, 256B)
E2 = 2 * E           # paired gather width
GCH = 1024           # idxs per merged gather call (hard SWDGE limit)
NGC = MN // GCH      # 5 merged gather calls per block
IGM = MN // 16       # 640 idx cols per block for the merged stream
ZP = 2 * ZN          # cen idxs gathered per BLOCK-PAIR (one 1024-idx gather)
IGZ2 = ZP // 16      # 64 cen idx cols per block-pair
IGP = 2 * IGM + IGZ2  # 1344 idx cols per block-pair
MG = MQ + NSB        # 84 mask cols per block
MARGIN = 1.0

_CACHE: dict = {}


def _wrap_idx(flat):
    """int16 idx list -> [128, n/16] wrapped-16, replicated across cores."""
    n = len(flat)
    nf = -(-n // 16)
    w = np.zeros((16, nf), np.int16)
    w[np.arange(n) % 16, np.arange(n) // 16] = flat
    return np.tile(w, (8, 1))


def _build_program():
    nc = bacc.Bacc("TRN2", target_bir_lowering=False, debug=False,
                   num_swdge_queues=4)

    gt_d = nc.dram_tensor("gt", [V, E], BF16, kind="ExternalInput")
    cu_d = nc.dram_tensor("cu", [V, E], BF16, kind="ExternalInput")
    wf_d = nc.dram_tensor("wf", [128, L + 1], BF16, kind="ExternalInput")
    idx_d = nc.dram_tensor("idx", [128, (NGB // 2) * IGP], dt.int16, kind="ExternalInput")
    msk_d = nc.dram_tensor("msk", [128, NGB * MG], dt.uint8, kind="ExternalInput")
    out_d = nc.dram_tensor("out", [128, 2], F32, kind="ExternalOutput")

    # overlapping paired views: pair k -> rows {2k, 2k+1} (512B)
    gt_v = bass.AP(gt_d, 0, [[E2, V // 2], [1, E2]])
    cu_v = bass.AP(cu_d, 0, [[E2, V // 2], [1, E2]])

    nregs = {}

    def gather(out_ap, tab_v, idx_ap, n):
        if n not in nregs:
            nregs[n] = nc.gpsimd.to_reg(n)
        nc.gpsimd.dma_gather(
            out_ap=out_ap, in_ap=tab_v, idxs_ap=idx_ap,
            num_idxs=n, num_idxs_reg=nregs[n], elem_size=E2, elem_step=E2,
            queue_num=0)

    with tile.TileContext(nc) as tc, ExitStack() as ctx:
        const = ctx.enter_context(tc.tile_pool(name="const", bufs=1))
        io = ctx.enter_context(tc.tile_pool(name="io", bufs=2))
        wk = ctx.enter_context(tc.tile_pool(name="wk", bufs=2))
        ps = ctx.enter_context(tc.tile_pool(name="ps", bufs=2, space="PSUM"))
        accp = ctx.enter_context(tc.tile_pool(name="accp", bufs=1))

        ident = const.tile([128, 128], BF16)
        make_identity(nc, ident[:])
        # idx uploaded per gather block so the first gathers start immediately
        idx_sb = const.tile([128, (NGB // 2) * IGP], dt.int16)
        idx_ap = idx_d.ap()
        nc.sync.dma_start(idx_sb[:, 0:IGP], idx_ap[:, 0:IGP])
        msk_sb = const.tile([128, NGB * MG], dt.uint8)
        nc.sync.dma_start(msk_sb[:], msk_d.ap())
        wf_sb = const.tile([128, L + 1], BF16)
        nc.sync.dma_start(wf_sb[:], wf_d.ap())
        for pi in range(1, NGB // 2):
            nc.sync.dma_start(idx_sb[:, pi * IGP:(pi + 1) * IGP],
                              idx_ap[:, pi * IGP:(pi + 1) * IGP])

        acc_h = accp.tile([128, NSB, C], F32)
        acc_c = accp.tile([128, NSB], F32)
        nc.vector.memset(acc_h[:], 0.0)
        nc.vector.memset(acc_c[:], 0.0)

        CG2 = None
        for gb in range(NGB):
            pi, half = divmod(gb, 2)
            PGN = io.tile([128, MQ, E2], BF16, tag="PGN")
            if half == 0:
                CG2 = io.tile([128, 2 * NSB, E2], BF16, tag="CG2")
                gather(CG2[:], cu_v,
                       idx_sb[:, pi * IGP + 2 * IGM:(pi + 1) * IGP], ZP)
            CG = CG2[:][:, half * NSB:(half + 1) * NSB, :]

            icol = pi * IGP + half * IGM
            for g in range(NGC):
                gather(PGN[:, g * (GCH // 128):(g + 1) * (GCH // 128), :], gt_v,
                       idx_sb[:, icol + g * (GCH // 16):icol + (g + 1) * (GCH // 16)],
                       GCH)

            # parity select on u32 views: keep the chosen 256B half's
            # useful 104 bf16 (52 u32) in cols [0:104)
            p32 = PGN[:].bitcast(U32)
            c32 = CG.bitcast(U32)
            mm = msk_sb[:, gb * MG:gb * MG + MQ]
            mz = msk_sb[:, gb * MG + MQ:(gb + 1) * MG]
            nc.vector.copy_predicated(
                p32[:, :, 0:26], mm.unsqueeze(2).to_broadcast([128, MQ, 26]),
                p32[:, :, 64:90])
            nc.vector.copy_predicated(
                c32[:, :, 0:26], mz.unsqueeze(2).to_broadcast([128, NSB, 26]),
                c32[:, :, 64:90])

            P4 = PGN[:].rearrange("p (s u) e -> p s u e", s=NSB)  # [128,s,20,256]

            # h = sum_j relu(U[ctx_j] + Ucen[cen]): tree-add over j
            y = wk.tile([128, NSB, C, D], BF16, tag="y")
            nc.vector.tensor_tensor(
                out=y[:], in0=P4[:, :, 0:C, 0:D],
                in1=CG[:, :, 0:D].unsqueeze(2).to_broadcast([128, NSB, C, D]),
                op=OP.add)
            r = wk.tile([128, NSB, C, D], BF16, tag="r")
            nc.scalar.activation(r[:], y[:], AF.Relu)
            t1 = wk.tile([128, NSB, 5, D], BF16, tag="t1")
            nc.vector.tensor_tensor(out=t1[:], in0=r[:, :, 0:5], in1=r[:, :, 5:10],
                                    op=OP.add)
            t2 = wk.tile([128, NSB, 2, D], BF16, tag="t2")
            nc.vector.tensor_tensor(out=t2[:], in0=t1[:, :, 0:2], in1=t1[:, :, 2:4],
                                    op=OP.add)
            t3 = wk.tile([128, NSB, D], BF16, tag="t3")
            nc.vector.tensor_tensor(out=t3[:], in0=t2[:, :, 0], in1=t2[:, :, 1],
                                    op=OP.add)
            h_all = wk.tile([128, NSB, 64], BF16, tag="h")
            nc.vector.memset(h_all[:, :, D:64], 0.0)
            nc.vector.memset(h_all[:, :, H:H + 1], 1.0)
            nc.vector.tensor_tensor(out=h_all[:, :, 0:D], in0=t3[:], in1=t1[:, :, 4],
                                    op=OP.add)

            A_t = wk.tile([128, NSB], F32, tag="A")
            musq_t = wk.tile([128, NSB], F32, tag="musq")
            lsg_t = wk.tile([128, NSB], F32, tag="lsg")
            sqj = wk.tile([128, L], BF16, tag="sqj")

            for sp in range(NSB // 2):
                hT_ps = ps.tile([128, 128], BF16, tag="hTp")
                nc.tensor.transpose(
                    hT_ps[:], h_all[:, 2 * sp:2 * sp + 2, :].rearrange("p a b -> p (a b)"),
                    ident[:])
                hT = wk.tile([128, 128], BF16, tag="hT")
                nc.scalar.copy(hT[:], hT_ps[:])
                for k in range(2):
                    s = 2 * sp + k
                    mu_ps = ps.tile([128, L + 1], F32, tag="mu")
                    nc.tensor.matmul(mu_ps[:], lhsT=hT[64 * k:64 * k + H + 1, :],
                                     rhs=wf_sb[64 * k:64 * k + H + 1, :],
                                     start=True, stop=True)
                    nc.scalar.activation(A_t[:, s:s + 1], mu_ps[:, L:L + 1], AF.Exp)
                    nc.scalar.activation(sqj[:], mu_ps[:, 0:L], AF.Square,
                                         accum_out=musq_t[:, s:s + 1])
                    nc.scalar.copy(lsg_t[:, s:s + 1], mu_ps[:, L:L + 1])

            AB = wk.tile([128, NSB], F32, tag="AB")
            nc.vector.tensor_tensor(out=AB[:], in0=A_t[:], in1=musq_t[:], op=OP.add)

            # dots: one big mult + segmented reduce over the merged stream
            z = wk.tile([128, NSB, SLOT, D], BF16, tag="z")
            nc.vector.tensor_tensor(
                out=z[:],
                in0=h_all[:, :, 0:D].unsqueeze(2).to_broadcast([128, NSB, SLOT, D]),
                in1=P4[:, :, :, D:2 * D], op=OP.mult)
            dots = wk.tile([128, NSB, SLOT], F32, tag="dots")
            nc.vector.tensor_reduce(out=dots[:], in_=z[:], axis=AX.X, op=OP.add)
            zc = wk.tile([128, NSB, D], BF16, tag="zc")
            nc.vector.tensor_tensor(out=zc[:], in0=h_all[:, :, 0:D],
                                    in1=CG[:, :, D:2 * D], op=OP.mult)
            dc = wk.tile([128, NSB], F32, tag="dc")
            nc.vector.tensor_reduce(out=dc[:], in_=zc[:], axis=AX.X, op=OP.add)

            # w = (exp(lsg) + musq + dot + s1)*iv + lv   per merged slot
            w = wk.tile([128, NSB, SLOT], F32, tag="w")
            nc.vector.tensor_tensor(
                out=w[:], in0=dots[:],
                in1=AB[:].unsqueeze(2).to_broadcast([128, NSB, SLOT]), op=OP.add)
            nc.vector.tensor_tensor(out=w[:], in0=w[:], in1=P4[:, :, :, 2 * D],
                                    op=OP.add)
            nc.vector.tensor_tensor(out=w[:], in0=w[:], in1=P4[:, :, :, 2 * D + 2],
                                    op=OP.mult)
            nc.vector.tensor_tensor(out=w[:], in0=w[:], in1=P4[:, :, :, 2 * D + 1],
                                    op=OP.add)
            d = wk.tile([128, NSB, C], F32, tag="d")
            nc.vector.tensor_tensor(out=d[:], in0=w[:, :, 0:C], in1=w[:, :, C:SLOT],
                                    op=OP.subtract)
            hng = wk.tile([128, NSB, C], F32, tag="hng")
            nc.scalar.activation(hng[:], d[:], AF.Relu, bias=float(MARGIN), scale=0.5)
            nc.vector.tensor_tensor(out=acc_h[:], in0=acc_h[:], in1=hng[:], op=OP.add)

            wc = wk.tile([128, NSB], F32, tag="wc")
            nc.vector.tensor_tensor(out=wc[:], in0=dc[:], in1=AB[:], op=OP.add)
            nc.vector.tensor_tensor(out=wc[:], in0=wc[:], in1=CG[:, :, 2 * D], op=OP.add)
            nc.vector.tensor_tensor(out=wc[:], in0=wc[:], in1=CG[:, :, 2 * D + 2],
                                    op=OP.mult)
            nc.vector.tensor_tensor(out=wc[:], in0=wc[:], in1=CG[:, :, 2 * D + 1],
                                    op=OP.add)
            nc.vector.tensor_tensor(out=wc[:], in0=wc[:], in1=lsg_t[:], op=OP.subtract)
            nc.vector.tensor_tensor(out=acc_c[:], in0=acc_c[:], in1=wc[:], op=OP.add)

        outt = accp.tile([128, 2], F32)
        nc.vector.tensor_reduce(out=outt[:, 0:1],
                                in_=acc_h[:].rearrange("p s u -> p (s u)"),
                                axis=AX.X, op=OP.add)
        nc.vector.tensor_reduce(out=outt[:, 1:2], in_=acc_c[:], axis=AX.X, op=OP.add)
        nc.sync.dma_start(out_d.ap(), outt[:])

    # Spread gathers across the 4 SWDGE queues (4 Q7 core-pairs run desc-gen
    # in parallel). queue = Tile-assigned DMASW sem lane % 4 keeps per-lane
    # completion FIFO within its queue, so Tile's sem ordering stays sound.
    import re
    for inst in nc.inst_map.values():
        if type(inst).__name__ == "InstDMAGatherAnt" and inst.sync_info:
            for u in inst.sync_info.on_update:
                m = re.match(r"DMASW(\d+)_", u.ant_name or "")
                if m:
                    inst.queue_num = int(m.group(1)) % 4
                    break
    nc.compile()
    return nc


def _prep_inputs(emb, W1, b1, Wmu, bmu, Wls, bls, type_means_tbl,
                 type_logvars_tbl, centers, contexts, neg_contexts):
    emb = np.asarray(emb, np.float32)
    W1 = np.asarray(W1, np.float32)
    Wmu = np.asarray(Wmu, np.float32)
    bmu = np.asarray(bmu, np.float32)
    tm = np.asarray(type_means_tbl, np.float32)
    lv = np.asarray(type_logvars_tbl, np.float32)[:, 0]

    U = emb @ W1[:D]
    Ucen = emb @ W1[D:] + np.asarray(b1, np.float32)
    Wp = -2.0 * (tm @ Wmu.T)
    s1 = (tm * tm).sum(axis=1) - 2.0 * (tm @ bmu)
    iv = np.exp(-lv)

    gt = np.zeros((V, E), np.float32)
    gt[:, 0:D] = U
    gt[:, D:2 * D] = Wp
    gt[:, 2 * D] = s1
    gt[:, 2 * D + 1] = lv
    gt[:, 2 * D + 2] = iv
    cu = gt.copy()
    cu[:, 0:D] = Ucen
    gt = gt.astype(ml_dtypes.bfloat16)
    cu = cu.astype(ml_dtypes.bfloat16)

    wf = np.zeros((128, L + 1), np.float32)
    for q in (0, 64):
        wf[q:q + H, 0:L] = Wmu
        wf[q:q + H, L] = np.asarray(Wls, np.float32)[:, 0]
        wf[q + H, 0:L] = bmu
        wf[q + H, L] = np.asarray(bls, np.float32)[0]
    wf = wf.astype(ml_dtypes.bfloat16)

    # merged ctx|neg stream: position n = ((s*2 + t)*C + j)*128 + p
    cx = np.asarray(contexts, np.int32).reshape(NCORES, NGB, NSB, 128, C)
    ng = np.asarray(neg_contexts, np.int32).reshape(NCORES, NGB, NSB, 128, C)
    cn = np.asarray(centers, np.int32).reshape(NCORES, NGB, NSB, 128)
    mg = np.stack([cx.transpose(0, 1, 2, 4, 3), ng.transpose(0, 1, 2, 4, 3)],
                  axis=3)                                 # [core,gb,s,t,j,p]
    mgf = mg.reshape(NCORES, NGB, MN)
    cnf = cn.reshape(NCORES, NGB, ZN)

    in_maps = []
    for c in range(NCORES):
        iparts, mparts = [], []
        for gb in range(NGB):
            f, fz = mgf[c, gb], cnf[c, gb]
            iparts.append(_wrap_idx((f >> 1).astype(np.int16)))
            if gb % 2 == 1:
                fz2 = np.concatenate([cnf[c, gb - 1], fz])
                iparts.append(_wrap_idx((fz2 >> 1).astype(np.int16)))
            mparts.append(np.ascontiguousarray(
                (f & 1).reshape(MQ, 128).T.astype(np.uint8)))
            mparts.append(np.ascontiguousarray(
                (fz & 1).reshape(NSB, 128).T.astype(np.uint8)))
        in_maps.append({
            "gt": gt, "cu": cu, "wf": wf,
            "idx": np.concatenate(iparts, axis=1),
            "msk": np.concatenate(mparts, axis=1),
        })
    return in_maps


def kernel(**inputs) -> np.ndarray:
    if "nc" not in _CACHE:
        _CACHE["nc"] = _build_program()
    nc = _CACHE["nc"]
    in_maps = _prep_inputs(**inputs)
    res = run_bass_kernel_spmd(nc, in_maps, core_ids=list(range(NCORES)))
    total = 0.0
    for c in range(NCORES):
        out = np.asarray(res.results[c]["out"], np.float64)
        total += out[:, 0].sum() + 0.5 * out[:, 1].sum()
    loss = total / B - L / 2.0
    return np.float32(loss)
